# revision 11
# baseline (speedup 1.0000x reference)
"""Trainium2 Bass kernel for CompositionalPhoneticsModel (segment_reduce).

Computation (reference):
    phone   = einsum('bth,hp->btp', enc_output, feature2phone) / sqrt(H)
    allo    = where(mapping>0, phone[:,:,None,:]*mapping, -inf)   # mapping is 0/1
    phoneme = max(allo, axis=-1)                                  # masked segment max
    out     = log_softmax(phoneme, axis=2)

Device strategy (8 NeuronCores, data-parallel over the B*T=8192 rows):
  * Host gathers feature2phone columns into segment-contiguous order
    (phones in 2 segments get duplicated columns; NNZ ~ 506) and sorts
    segments by length so the per-segment max is a handful of strided DVE
    reduce_max ops.  Host un-permutes the output columns at the end.
  * W ships as fp8 e3m4 (range +-15.5, 4 mantissa bits: ~2x the quantization
    noise of bf16's 8 bits on N(0,1) data but only ~0.8% of the output error
    budget).  The 1/sqrt(H) scale is NOT folded in (it would push W into the
    e3m4 denormal range); it rides the Exp activation's scale input and a
    fused MULT in the final tensor_scalar.  enc stays bf16 - its DMA
    overlaps the matmul stream anyway, so quantizing it buys nothing.
  * DMA on this part is bandwidth-bound at ~287 GB/s once packets are
    >=2KB/partition, with ~2.4us from program start to first byte.  The
    first matmul gates on W + enc block 0, so W (0.32MB in fp8) goes first,
    then enc in pieces sized so the stream stays ahead of the PE: blocks
    [0], [1], [2-3], [4-7].  Output: blocks 0-5 stored during the last
    megatile's compute; the last 2 blocks split by partition halves across
    two queues right after their log-softmax.
  * Postlude per 256-row megatile: batched strided segment-max reduces
    (DVE), per-128-row Exp on ScalarE with the row-sum from the activation
    accumulator, one Ln, and a fused (x*scale - lse) DVE tensor_scalar.
    The last megatile runs everything per 128-row block instead, shortening
    the serial tail after the final matmul.
"""

from contextlib import ExitStack

import numpy as np
import ml_dtypes

import concourse.bass as bass
import concourse.bacc as bacc
import concourse.tile as tile
from concourse import mybir
from concourse.bass_utils import run_bass_kernel_spmd

B, T, H = 8, 1024, 640
N_PHONEME, N_PHONE = 96, 230
N_CORES = 8
ROWS = B * T
RC = ROWS // N_CORES          # rows per core
NH = H // 128                 # contraction chunks
NB = RC // 128                # 128-row blocks per core
NMT = NB // 2                 # megatiles (2 blocks each)
BF16 = ml_dtypes.bfloat16
F8 = ml_dtypes.float8_e3m4
SCALE = float(1.0 / np.sqrt(np.float32(H)))


def _structure(mapping: np.ndarray):
    """Segment-contiguous gather order, grouped by segment length (desc).

    Returns (col_ids, groups, perm):
      col_ids: phone index feeding each device matmul column (len NNZ)
      groups:  list of (L, nL, col_off, out_off) — nL segments of length L
               occupy matmul cols [col_off, col_off+nL*L) and device output
               cols [out_off, out_off+nL)
      perm:    perm[j] = original phoneme id of device output column j
    """
    segs = [np.nonzero(mapping[m] > 0)[0] for m in range(N_PHONEME)]
    assert min(len(s) for s in segs) >= 1
    # pad segment lengths up to even targets (repeating a member doesn't
    # change the max): fewer distinct lengths -> fewer DVE reduce ops.
    # Only worthwhile while the matmul width stays within one PSUM bank.
    padded = []
    for s in segs:
        t = ((len(s) + 1) // 2) * 2
        padded.append(np.concatenate([s, np.full(t - len(s), s[0], s.dtype)]))
    if sum(len(s) for s in padded) <= 512:
        segs = padded
    lengths = np.array([len(s) for s in segs])
    order = np.argsort(-lengths, kind="stable")
    col_ids, groups, perm = [], [], []
    i = 0
    while i < N_PHONEME:
        L = int(lengths[order[i]])
        j = i
        while j < N_PHONEME and lengths[order[j]] == L:
            j += 1
        groups.append((L, j - i, len(col_ids), i))
        for k in range(i, j):
            m = int(order[k])
            col_ids.extend(segs[m].tolist())
            perm.append(m)
        i = j
    return np.array(col_ids, dtype=np.int64), groups, np.array(perm, dtype=np.int64)


def _patch_act_tables():
    """Make Exp and Ln resolve to the same activation-table set.

    bacc's insert_act_table_loads models a single table slot, so a kernel
    alternating Exp/Ln reloads a 1.3us table on every transition.  act_info
    has a joint set ('natural_log_exp_and_others') containing both; keep the
    set list's order/indices intact but strip Exp/Ln from the other sets so
    the pass picks the joint set for both and emits a single load.
    """
    if getattr(bacc, "_act_tables_patched", False):
        return
    from concourse import hw_specs
    orig = hw_specs.get_activation_tables
    act = mybir.ActivationFunctionType

    def patched(module_arch):
        tabs = orig(module_arch)
        joint = [k for k, v in tabs.items() if act.Exp in v and act.Ln in v]
        if not joint:
            return tabs
        j = joint[0]
        return {
            k: (v if k == j else (v - {act.Exp, act.Ln}))
            for k, v in tabs.items()
        }

    bacc.get_activation_tables = patched
    bacc._act_tables_patched = True


def _build_program(nnz: int, groups):
    """Build + compile the per-core Bass program. Returns the Bacc object."""
    _patch_act_tables()
    nc = bacc.Bacc("TRN2", target_bir_lowering=False, debug=False)
    dt = mybir.dt
    act = mybir.ActivationFunctionType
    X = mybir.AxisListType.X
    alu = mybir.AluOpType

    # enc interleaved: [128, RC, NH]; element (p, r, c) = enc[r, c*128+p]
    enck_d = nc.dram_tensor("enck", [128, RC, NH], dt.bfloat16, kind="ExternalInput")
    # W interleaved: [128, NH, nnz]; element (p, c, n) = W[c*128+p, n], fp8
    wk_d = nc.dram_tensor("wk", [128, NH, nnz], dt.float8e3, kind="ExternalInput")
    # out packed: [128, NB, 96]; element (p, b, m) = out[b*128+p, m]
    out_d = nc.dram_tensor("out", [128, NB, N_PHONEME], dt.float32, kind="ExternalOutput")

    with ExitStack() as ctx:
        tc = ctx.enter_context(tile.TileContext(nc))
        wpool = ctx.enter_context(tc.tile_pool(name="wpool", bufs=1))
        epool = ctx.enter_context(tc.tile_pool(name="epool", bufs=1))
        # 3 double-bank megatile accumulators + 2 single-bank ones for the
        # last megatile (separate tiles per 128-row block there, so its r0
        # postlude reads never alias r1's accumulation in the hazard
        # tracker - a shared tile serializes the PE behind the DVE)
        ppool = ctx.enter_context(tc.tile_pool(name="ppool", bufs=3, space="PSUM"))
        ppool2 = ctx.enter_context(tc.tile_pool(name="ppool2", bufs=2, space="PSUM"))
        spool = ctx.enter_context(tc.tile_pool(name="spool", bufs=3))
        opool = ctx.enter_context(tc.tile_pool(name="opool", bufs=1))

        # Loads, strictly ordered on the Sync queue (the stream is
        # bandwidth-bound, so order = priority): W gates the first matmul,
        # then enc pieces sized to stay ahead of the PE.
        wt = wpool.tile([128, NH, nnz], dt.float8e3)
        nc.sync.dma_start(wt[:], wk_d[:])
        et = epool.tile([128, RC, NH], dt.bfloat16)
        for lo, hi in ((0, 128), (128, 256), (256, 512), (512, RC)):
            nc.sync.dma_start(et[:, lo:hi, :], enck_d[:, lo:hi, :])

        # PE warmup: dummy matmuls on zeroed scratch run while the DMAs
        # land, ramping the tensor engine's p-state.  They write the first
        # megatile's PSUM bank; the real accumulation overwrites it.
        wu = wpool.tile([128, 512], dt.bfloat16)
        nc.vector.memset(wu[:], 0.0)
        ps0 = ppool.tile([128, 2, 512], dt.float32, tag="ps")
        for _ in range(7):
            nc.tensor.matmul(ps0[:, 0, :], wu[:, :128], wu[:], start=True, stop=True)

        obuf = opool.tile([128, NB, N_PHONEME], dt.float32)

        def seg_max(ps, rr, nr, tagn):
            """Segment max of PSUM rows `rr`: one strided DVE reduce per
            segment-length group.  (A pairwise tensor_tensor pre-max would
            halve the reduce work, but the DVE can read only ONE operand
            from PSUM per instruction - walrus NCC_IBVF027.)"""
            pmax = spool.tile([128, nr, N_PHONEME], dt.float32, tag=f"pmax{tagn}")
            for (L, nL, coff, ooff) in groups:
                src = ps[:, rr, coff:coff + nL * L].rearrange(
                    "p r (s l) -> p r s l", l=L
                )
                nc.vector.reduce_max(pmax[:, :, ooff:ooff + nL], src, axis=X)
            return pmax

        def postlude(ps, rr, blk, nr):
            """log-softmax for `nr` row blocks starting at `blk`."""
            pmax = seg_max(ps, rr, nr, nr)
            ex = spool.tile([128, nr, N_PHONEME], dt.float32, tag=f"ex{nr}")
            se = spool.tile([128, nr], dt.float32, tag=f"se{nr}")
            for k in range(nr):
                # exp(scale*x); the row-sum comes free via the activation
                # accumulator (1/sqrt(H) lives here, not in fp8 W)
                nc.scalar.activation(ex[:, k, :], pmax[:, k, :], act.Exp,
                                     scale=SCALE, accum_out=se[:, k:k + 1])
            lse = spool.tile([128, nr], dt.float32, tag=f"lse{nr}")
            nc.scalar.activation(lse[:], se[:], act.Ln)
            for k in range(nr):
                # out = scale*pmax - lse, fused in one DVE op
                nc.vector.tensor_scalar(
                    obuf[:, blk + k, :], pmax[:, k, :],
                    SCALE, lse[:, k:k + 1], op0=alu.mult, op1=alu.subtract,
                )

        def block_matmuls(ps_row, blk):
            row0 = blk * 128
            for c in range(NH):
                nc.tensor.matmul(
                    ps_row[:, :nnz],
                    et[:, row0:row0 + 128, c],
                    wt[:, c, :],
                    start=(c == 0),
                    stop=(c == NH - 1),
                )

        for mt in range(NMT - 1):
            ps = ps0 if mt == 0 else ppool.tile([128, 2, 512], dt.float32, tag="ps")
            for r in range(2):
                block_matmuls(ps[:, r, :], mt * 2 + r)
            postlude(ps, slice(0, 2), mt * 2, 2)
            if mt == NMT - 2:
                # blocks 0-5 stored while the last megatile computes
                nc.sync.dma_start(out_d[:, :6, :], obuf[:, :6, :])
        # last megatile: one single-bank tile per block, postlude per block
        # right after that block's accumulation - the DVE chews block 6
        # while the PE streams block 7, and the tail after the final matmul
        # is one block's postlude, not two.
        psr0 = ppool2.tile([128, 1, 512], dt.float32, tag="psr")
        psr1 = ppool2.tile([128, 1, 512], dt.float32, tag="psr")
        block_matmuls(psr0[:, 0, :], NB - 2)
        pm6 = seg_max(psr0, slice(0, 1), 1, "t6")
        block_matmuls(psr1[:, 0, :], NB - 1)
        pm7 = seg_max(psr1, slice(0, 1), 1, "t7")
        # tail postlude: row-sums on the (now idle) DVE instead of the
        # activation-accumulator round-trip, and a single Ln for both
        # blocks - 3 ScalarE ops on the post-stream critical path
        exL = spool.tile([128, 2, N_PHONEME], dt.float32, tag="exL")
        seL = spool.tile([128, 2], dt.float32, tag="seL")
        nc.scalar.activation(exL[:, 0, :], pm6[:, 0, :], act.Exp, scale=SCALE)
        nc.vector.reduce_sum(seL[:, 0:1], exL[:, 0:1, :], axis=X)
        nc.scalar.activation(exL[:, 1, :], pm7[:, 0, :], act.Exp, scale=SCALE)
        nc.vector.reduce_sum(seL[:, 1:2], exL[:, 1:2, :], axis=X)
        lseL = spool.tile([128, 2], dt.float32, tag="lseL")
        nc.scalar.activation(lseL[:], seL[:], act.Ln)
        nc.vector.tensor_scalar(obuf[:, NB - 2, :], pm6[:, 0, :],
                                SCALE, lseL[:, 0:1], op0=alu.mult, op1=alu.subtract)
        nc.vector.tensor_scalar(obuf[:, NB - 1, :], pm7[:, 0, :],
                                SCALE, lseL[:, 1:2], op0=alu.mult, op1=alu.subtract)
        # final piece: blocks 6-7, split by partition halves onto two
        # queues (sync + scalar) so the packets can spread across engines
        nc.sync.dma_start(out_d[:64, 6:, :], obuf[:64, 6:, :])
        nc.scalar.dma_start(out_d[64:, 6:, :], obuf[64:, 6:, :])

    nc.compile()
    return nc


_CACHE: dict = {}


def _get_compiled(mapping: np.ndarray):
    key = mapping.astype(np.float32).tobytes()
    if _CACHE.get("key") != key:
        col_ids, groups, perm = _structure(mapping)
        nc = _build_program(len(col_ids), groups)
        _CACHE.update(key=key, col_ids=col_ids, groups=groups, perm=perm, nc=nc)
    return _CACHE["nc"], _CACHE["col_ids"], _CACHE["perm"]


def _prep_in_maps(enc_output, feature2phone, col_ids):
    wg = feature2phone.astype(np.float32)[:, col_ids].astype(F8)
    # [H, nnz] -> [128, NH, nnz]
    wk = np.ascontiguousarray(wg.reshape(NH, 128, -1).transpose(1, 0, 2))
    # enc [ROWS, H] -> [128, ROWS, NH]
    e3 = enc_output.astype(BF16).reshape(ROWS, NH, 128)
    enck = np.ascontiguousarray(e3.transpose(2, 0, 1))
    in_maps = []
    for c in range(N_CORES):
        in_maps.append({
            "enck": np.ascontiguousarray(enck[:, c * RC:(c + 1) * RC, :]),
            "wk": wk,
        })
    return in_maps


def run_device(enc_output, feature2phone, mapping, trace=False, **kw):
    """Build/compile (cached), run on the 8 cores, return (output, BassKernelResults)."""
    enc_output = np.asarray(enc_output)
    feature2phone = np.asarray(feature2phone)
    mapping = np.asarray(mapping)
    nc, col_ids, perm = _get_compiled(mapping)
    in_maps = _prep_in_maps(enc_output, feature2phone, col_ids)
    res = run_bass_kernel_spmd(
        nc, in_maps, core_ids=list(range(N_CORES)), trace=trace, **kw
    )
    # device out [128, NB, 96] packed -> rows b*128+p
    dev = np.concatenate(
        [res.results[c]["out"].transpose(1, 0, 2).reshape(RC, N_PHONEME)
         for c in range(N_CORES)],
        axis=0,
    )
    out = np.empty_like(dev)
    out[:, perm] = dev
    return out.reshape(B, T, N_PHONEME).astype(np.float32), res


def kernel(enc_output, feature2phone, mapping):
    out, _ = run_device(enc_output, feature2phone, mapping)
    return out


# revision 13
# speedup vs baseline: 1.0211x; 1.0211x over previous
"""Trainium2 Bass kernel for CompositionalPhoneticsModel (segment_reduce).

Computation (reference):
    phone   = einsum('bth,hp->btp', enc_output, feature2phone) / sqrt(H)
    allo    = where(mapping>0, phone[:,:,None,:]*mapping, -inf)   # mapping is 0/1
    phoneme = max(allo, axis=-1)                                  # masked segment max
    out     = log_softmax(phoneme, axis=2)

Device strategy (8 NeuronCores, data-parallel over the B*T=8192 rows):
  * Host gathers feature2phone columns into segment-contiguous order
    (phones in 2 segments get duplicated columns; NNZ ~ 506) and sorts
    segments by length so the per-segment max is a handful of strided DVE
    reduce_max ops.  Host un-permutes the output columns at the end.
  * W ships as fp8 e3m4 (range +-15.5, 4 mantissa bits: ~2x the quantization
    noise of bf16's 8 bits on N(0,1) data but only ~0.8% of the output error
    budget).  The 1/sqrt(H) scale is NOT folded in (it would push W into the
    e3m4 denormal range); it rides the Exp activation's scale input and a
    fused MULT in the final tensor_scalar.  enc stays bf16 - its DMA
    overlaps the matmul stream anyway, so quantizing it buys nothing.
  * DMA on this part is bandwidth-bound at ~287 GB/s once packets are
    >=2KB/partition, with ~2.4us from program start to first byte.  The
    first matmul gates on W + enc block 0, so W (0.32MB in fp8) goes first,
    then enc in pieces sized so the stream stays ahead of the PE: blocks
    [0], [1], [2-3], [4-7].  Output: blocks 0-5 stored during the last
    megatile's compute; the last 2 blocks split by partition halves across
    two queues right after their log-softmax.
  * Postlude per 256-row megatile: batched strided segment-max reduces
    (DVE), per-128-row Exp on ScalarE with the row-sum from the activation
    accumulator, one Ln, and a fused (x*scale - lse) DVE tensor_scalar.
    The last megatile runs everything per 128-row block instead, shortening
    the serial tail after the final matmul.
"""

from contextlib import ExitStack

import numpy as np
import ml_dtypes

import concourse.bass as bass
import concourse.bacc as bacc
import concourse.tile as tile
from concourse import mybir
from concourse.bass_utils import run_bass_kernel_spmd

B, T, H = 8, 1024, 640
N_PHONEME, N_PHONE = 96, 230
N_CORES = 8
ROWS = B * T
RC = ROWS // N_CORES          # rows per core
NH = H // 128                 # contraction chunks
NB = RC // 128                # 128-row blocks per core
NMT = NB // 2                 # megatiles (2 blocks each)
BF16 = ml_dtypes.bfloat16
F8 = ml_dtypes.float8_e3m4
SCALE = float(1.0 / np.sqrt(np.float32(H)))


def _structure(mapping: np.ndarray):
    """Segment-contiguous gather order, grouped by segment length (desc).

    Returns (col_ids, groups, perm):
      col_ids: phone index feeding each device matmul column (len NNZ)
      groups:  list of (L, nL, col_off, out_off) — nL segments of length L
               occupy matmul cols [col_off, col_off+nL*L) and device output
               cols [out_off, out_off+nL)
      perm:    perm[j] = original phoneme id of device output column j
    """
    segs = [np.nonzero(mapping[m] > 0)[0] for m in range(N_PHONEME)]
    assert min(len(s) for s in segs) >= 1
    # pad segment lengths up to even targets (repeating a member doesn't
    # change the max): fewer distinct lengths -> fewer DVE reduce ops.
    # Only worthwhile while the matmul width stays within one PSUM bank.
    padded = []
    for s in segs:
        t = ((len(s) + 1) // 2) * 2
        padded.append(np.concatenate([s, np.full(t - len(s), s[0], s.dtype)]))
    if sum(len(s) for s in padded) <= 512:
        segs = padded
    lengths = np.array([len(s) for s in segs])
    order = np.argsort(-lengths, kind="stable")
    col_ids, groups, perm = [], [], []
    i = 0
    while i < N_PHONEME:
        L = int(lengths[order[i]])
        j = i
        while j < N_PHONEME and lengths[order[j]] == L:
            j += 1
        groups.append((L, j - i, len(col_ids), i))
        for k in range(i, j):
            m = int(order[k])
            col_ids.extend(segs[m].tolist())
            perm.append(m)
        i = j
    return np.array(col_ids, dtype=np.int64), groups, np.array(perm, dtype=np.int64)


def _patch_act_tables():
    """Make Exp and Ln resolve to the same activation-table set.

    bacc's insert_act_table_loads models a single table slot, so a kernel
    alternating Exp/Ln reloads a 1.3us table on every transition.  act_info
    has a joint set ('natural_log_exp_and_others') containing both; keep the
    set list's order/indices intact but strip Exp/Ln from the other sets so
    the pass picks the joint set for both and emits a single load.
    """
    if getattr(bacc, "_act_tables_patched", False):
        return
    from concourse import hw_specs
    orig = hw_specs.get_activation_tables
    act = mybir.ActivationFunctionType

    def patched(module_arch):
        tabs = orig(module_arch)
        joint = [k for k, v in tabs.items() if act.Exp in v and act.Ln in v]
        if not joint:
            return tabs
        j = joint[0]
        return {
            k: (v if k == j else (v - {act.Exp, act.Ln}))
            for k, v in tabs.items()
        }

    bacc.get_activation_tables = patched
    bacc._act_tables_patched = True


def _build_program(nnz: int, groups):
    """Build + compile the per-core Bass program. Returns the Bacc object."""
    _patch_act_tables()
    nc = bacc.Bacc("TRN2", target_bir_lowering=False, debug=False)
    dt = mybir.dt
    act = mybir.ActivationFunctionType
    X = mybir.AxisListType.X
    alu = mybir.AluOpType

    # enc interleaved: [128, RC, NH]; element (p, r, c) = enc[r, c*128+p]
    enck_d = nc.dram_tensor("enck", [128, RC, NH], dt.bfloat16, kind="ExternalInput")
    # W interleaved: [128, NH, nnz]; element (p, c, n) = W[c*128+p, n], fp8
    wk_d = nc.dram_tensor("wk", [128, NH, nnz], dt.float8e3, kind="ExternalInput")
    # out packed: [128, NB, 96]; element (p, b, m) = out[b*128+p, m]
    out_d = nc.dram_tensor("out", [128, NB, N_PHONEME], dt.float32, kind="ExternalOutput")

    with ExitStack() as ctx:
        tc = ctx.enter_context(tile.TileContext(nc))
        wpool = ctx.enter_context(tc.tile_pool(name="wpool", bufs=1))
        epool = ctx.enter_context(tc.tile_pool(name="epool", bufs=1))
        # 3 double-bank megatile accumulators + 2 single-bank ones for the
        # last megatile (separate tiles per 128-row block there, so its r0
        # postlude reads never alias r1's accumulation in the hazard
        # tracker - a shared tile serializes the PE behind the DVE)
        ppool = ctx.enter_context(tc.tile_pool(name="ppool", bufs=3, space="PSUM"))
        ppool2 = ctx.enter_context(tc.tile_pool(name="ppool2", bufs=2, space="PSUM"))
        spool = ctx.enter_context(tc.tile_pool(name="spool", bufs=3))
        opool = ctx.enter_context(tc.tile_pool(name="opool", bufs=1))

        # Loads, strictly ordered on the Sync queue (the stream is
        # bandwidth-bound, so order = priority): W gates the first matmul,
        # then enc pieces sized to stay ahead of the PE.
        wt = wpool.tile([128, NH, nnz], dt.float8e3)
        nc.sync.dma_start(wt[:], wk_d[:])
        et = epool.tile([128, RC, NH], dt.bfloat16)
        for lo, hi in ((0, 128), (128, 256), (256, 512), (512, RC)):
            nc.sync.dma_start(et[:, lo:hi, :], enck_d[:, lo:hi, :])

        # PE warmup: dummy matmuls on zeroed scratch run while the DMAs
        # land, ramping the tensor engine's p-state.  They write the first
        # megatile's PSUM bank; the real accumulation overwrites it.
        wu = wpool.tile([128, 512], dt.bfloat16)
        nc.vector.memset(wu[:], 0.0)
        ps0 = ppool.tile([128, 2, 512], dt.float32, tag="ps")
        for _ in range(5):
            nc.tensor.matmul(ps0[:, 0, :], wu[:, :128], wu[:], start=True, stop=True)

        obuf = opool.tile([128, NB, N_PHONEME], dt.float32)

        def seg_max(ps, rr, nr, tagn):
            """Segment max of PSUM rows `rr`: one strided DVE reduce per
            segment-length group.  (A pairwise tensor_tensor pre-max would
            halve the reduce work, but the DVE can read only ONE operand
            from PSUM per instruction - walrus NCC_IBVF027.)"""
            pmax = spool.tile([128, nr, N_PHONEME], dt.float32, tag=f"pmax{tagn}")
            for (L, nL, coff, ooff) in groups:
                src = ps[:, rr, coff:coff + nL * L].rearrange(
                    "p r (s l) -> p r s l", l=L
                )
                nc.vector.reduce_max(pmax[:, :, ooff:ooff + nL], src, axis=X)
            return pmax

        def postlude_a(ps, rr, blk, nr):
            """Segment max + exp-sum + Ln for `nr` blocks at `blk`; the
            final subs are emitted LATER (postlude_b) so they queue behind
            the next megatile's reduces: a sub waits on this tile's Ln
            (~1us of ScalarE chain), and the DVE executes in order - subs
            emitted eagerly head-of-line block the next reduces."""
            pmax = seg_max(ps, rr, nr, nr if nr == 2 else blk)
            ex = spool.tile([128, nr, N_PHONEME], dt.float32, tag=f"ex{blk}")
            se = spool.tile([128, nr], dt.float32, tag=f"se{blk}")
            for k in range(nr):
                # exp(scale*x); the row-sum comes free via the activation
                # accumulator (1/sqrt(H) lives here, not in fp8 W)
                nc.scalar.activation(ex[:, k, :], pmax[:, k, :], act.Exp,
                                     scale=SCALE, accum_out=se[:, k:k + 1])
            lse = spool.tile([128, nr], dt.float32, tag=f"lse{blk}")
            nc.scalar.activation(lse[:], se[:], act.Ln)
            return pmax, lse

        def postlude_b(state, blk, nr):
            pmax, lse = state
            for k in range(nr):
                # out = scale*pmax - lse, fused in one DVE op
                nc.vector.tensor_scalar(
                    obuf[:, blk + k, :], pmax[:, k, :],
                    SCALE, lse[:, k:k + 1], op0=alu.mult, op1=alu.subtract,
                )

        def block_matmuls(ps_row, blk):
            row0 = blk * 128
            for c in range(NH):
                nc.tensor.matmul(
                    ps_row[:, :nnz],
                    et[:, row0:row0 + 128, c],
                    wt[:, c, :],
                    start=(c == 0),
                    stop=(c == NH - 1),
                )

        states = []
        for mt in range(NMT - 1):
            ps = ps0 if mt == 0 else ppool.tile([128, 2, 512], dt.float32, tag="ps")
            for r in range(2):
                block_matmuls(ps[:, r, :], mt * 2 + r)
            states.append(postlude_a(ps, slice(0, 2), mt * 2, 2))
            if mt > 0:
                postlude_b(states[mt - 1], (mt - 1) * 2, 2)
        # last megatile: one single-bank tile per block so neither block's
        # postlude aliases the other's accumulation in the hazard tracker
        psr0 = ppool2.tile([128, 1, 512], dt.float32, tag="psr")
        psr1 = ppool2.tile([128, 1, 512], dt.float32, tag="psr")
        block_matmuls(psr0[:, 0, :], NB - 2)
        st6 = postlude_a(psr0, slice(0, 1), NB - 2, 1)
        block_matmuls(psr1[:, 0, :], NB - 1)
        st7 = postlude_a(psr1, slice(0, 1), NB - 1, 1)
        postlude_b(states[NMT - 2], (NMT - 2) * 2, 2)
        # blocks 0-5 stored while the tail drains
        nc.sync.dma_start(out_d[:, :6, :], obuf[:, :6, :])
        postlude_b(st6, NB - 2, 1)
        postlude_b(st7, NB - 1, 1)
        # final piece: blocks 6-7, split by partition halves onto two
        # queues (sync + scalar) so the packets can spread across engines
        nc.sync.dma_start(out_d[:64, 6:, :], obuf[:64, 6:, :])
        nc.scalar.dma_start(out_d[64:, 6:, :], obuf[64:, 6:, :])

    nc.compile()
    return nc


_CACHE: dict = {}


def _get_compiled(mapping: np.ndarray):
    key = mapping.astype(np.float32).tobytes()
    if _CACHE.get("key") != key:
        col_ids, groups, perm = _structure(mapping)
        nc = _build_program(len(col_ids), groups)
        _CACHE.update(key=key, col_ids=col_ids, groups=groups, perm=perm, nc=nc)
    return _CACHE["nc"], _CACHE["col_ids"], _CACHE["perm"]


def _prep_in_maps(enc_output, feature2phone, col_ids):
    wg = feature2phone.astype(np.float32)[:, col_ids].astype(F8)
    # [H, nnz] -> [128, NH, nnz]
    wk = np.ascontiguousarray(wg.reshape(NH, 128, -1).transpose(1, 0, 2))
    # enc [ROWS, H] -> [128, ROWS, NH]
    e3 = enc_output.astype(BF16).reshape(ROWS, NH, 128)
    enck = np.ascontiguousarray(e3.transpose(2, 0, 1))
    in_maps = []
    for c in range(N_CORES):
        in_maps.append({
            "enck": np.ascontiguousarray(enck[:, c * RC:(c + 1) * RC, :]),
            "wk": wk,
        })
    return in_maps


def run_device(enc_output, feature2phone, mapping, trace=False, **kw):
    """Build/compile (cached), run on the 8 cores, return (output, BassKernelResults)."""
    enc_output = np.asarray(enc_output)
    feature2phone = np.asarray(feature2phone)
    mapping = np.asarray(mapping)
    nc, col_ids, perm = _get_compiled(mapping)
    in_maps = _prep_in_maps(enc_output, feature2phone, col_ids)
    res = run_bass_kernel_spmd(
        nc, in_maps, core_ids=list(range(N_CORES)), trace=trace, **kw
    )
    # device out [128, NB, 96] packed -> rows b*128+p
    dev = np.concatenate(
        [res.results[c]["out"].transpose(1, 0, 2).reshape(RC, N_PHONEME)
         for c in range(N_CORES)],
        axis=0,
    )
    out = np.empty_like(dev)
    out[:, perm] = dev
    return out.reshape(B, T, N_PHONEME).astype(np.float32), res


def kernel(enc_output, feature2phone, mapping):
    out, _ = run_device(enc_output, feature2phone, mapping)
    return out


# revision 14
# speedup vs baseline: 1.0680x; 1.0459x over previous
"""Trainium2 Bass kernel for CompositionalPhoneticsModel (segment_reduce).

Computation (reference):
    phone   = einsum('bth,hp->btp', enc_output, feature2phone) / sqrt(H)
    allo    = where(mapping>0, phone[:,:,None,:]*mapping, -inf)   # mapping is 0/1
    phoneme = max(allo, axis=-1)                                  # masked segment max
    out     = log_softmax(phoneme, axis=2)

Device strategy (8 NeuronCores, data-parallel over the B*T=8192 rows):
  * Host gathers feature2phone columns into segment-contiguous order
    (phones in 2 segments get duplicated columns; NNZ ~ 506) and sorts
    segments by length so the per-segment max is a handful of strided DVE
    reduce_max ops.  Host un-permutes the output columns at the end.
  * W ships as fp8 e3m4 (range +-15.5, 4 mantissa bits: ~2x the quantization
    noise of bf16's 8 bits on N(0,1) data but only ~0.8% of the output error
    budget).  The 1/sqrt(H) scale is NOT folded in (it would push W into the
    e3m4 denormal range); it rides the Exp activation's scale input and a
    fused MULT in the final tensor_scalar.  enc stays bf16 - its DMA
    overlaps the matmul stream anyway, so quantizing it buys nothing.
  * DMA on this part is bandwidth-bound at ~287 GB/s once packets are
    >=2KB/partition, with ~2.4us from program start to first byte.  The
    first matmul gates on W + enc block 0, so W (0.32MB in fp8) goes first,
    then enc in pieces sized so the stream stays ahead of the PE: blocks
    [0], [1], [2-3], [4-7].  Output: blocks 0-5 stored during the last
    megatile's compute; the last 2 blocks split by partition halves across
    two queues right after their log-softmax.
  * Postlude per 256-row megatile: batched strided segment-max reduces
    (DVE), per-128-row Exp on ScalarE with the row-sum from the activation
    accumulator, one Ln, and a fused (x*scale - lse) DVE tensor_scalar.
    The last megatile runs everything per 128-row block instead, shortening
    the serial tail after the final matmul.
"""

from contextlib import ExitStack

import numpy as np
import ml_dtypes

import concourse.bass as bass
import concourse.bacc as bacc
import concourse.tile as tile
from concourse import mybir
from concourse.bass_utils import run_bass_kernel_spmd

B, T, H = 8, 1024, 640
N_PHONEME, N_PHONE = 96, 230
N_CORES = 8
ROWS = B * T
RC = ROWS // N_CORES          # rows per core
NH = H // 128                 # contraction chunks
NB = RC // 128                # 128-row blocks per core
NMT = NB // 2                 # megatiles (2 blocks each)
BF16 = ml_dtypes.bfloat16
F8 = ml_dtypes.float8_e3m4
SCALE = float(1.0 / np.sqrt(np.float32(H)))


def _structure(mapping: np.ndarray):
    """Segment-contiguous gather order, grouped by segment length (desc).

    Returns (col_ids, groups, perm):
      col_ids: phone index feeding each device matmul column (len NNZ)
      groups:  list of (L, nL, col_off, out_off) — nL segments of length L
               occupy matmul cols [col_off, col_off+nL*L) and device output
               cols [out_off, out_off+nL)
      perm:    perm[j] = original phoneme id of device output column j
    """
    segs = [np.nonzero(mapping[m] > 0)[0] for m in range(N_PHONEME)]
    assert min(len(s) for s in segs) >= 1
    # pad segment lengths up to even targets (repeating a member doesn't
    # change the max): fewer distinct lengths -> fewer DVE reduce ops.
    # Only worthwhile while the matmul width stays within one PSUM bank.
    padded = []
    for s in segs:
        t = ((len(s) + 1) // 2) * 2
        padded.append(np.concatenate([s, np.full(t - len(s), s[0], s.dtype)]))
    if sum(len(s) for s in padded) <= 512:
        segs = padded
    lengths = np.array([len(s) for s in segs])
    order = np.argsort(-lengths, kind="stable")
    col_ids, groups, perm = [], [], []
    i = 0
    while i < N_PHONEME:
        L = int(lengths[order[i]])
        j = i
        while j < N_PHONEME and lengths[order[j]] == L:
            j += 1
        groups.append((L, j - i, len(col_ids), i))
        for k in range(i, j):
            m = int(order[k])
            col_ids.extend(segs[m].tolist())
            perm.append(m)
        i = j
    return np.array(col_ids, dtype=np.int64), groups, np.array(perm, dtype=np.int64)


def _patch_act_tables():
    """Make Exp and Ln resolve to the same activation-table set.

    bacc's insert_act_table_loads models a single table slot, so a kernel
    alternating Exp/Ln reloads a 1.3us table on every transition.  act_info
    has a joint set ('natural_log_exp_and_others') containing both; keep the
    set list's order/indices intact but strip Exp/Ln from the other sets so
    the pass picks the joint set for both and emits a single load.
    """
    if getattr(bacc, "_act_tables_patched", False):
        return
    from concourse import hw_specs
    orig = hw_specs.get_activation_tables
    act = mybir.ActivationFunctionType

    def patched(module_arch):
        tabs = orig(module_arch)
        joint = [k for k, v in tabs.items() if act.Exp in v and act.Ln in v]
        if not joint:
            return tabs
        j = joint[0]
        return {
            k: (v if k == j else (v - {act.Exp, act.Ln}))
            for k, v in tabs.items()
        }

    bacc.get_activation_tables = patched
    bacc._act_tables_patched = True


def _build_program(nnz: int, groups):
    """Build + compile the per-core Bass program. Returns the Bacc object."""
    _patch_act_tables()
    nc = bacc.Bacc("TRN2", target_bir_lowering=False, debug=False)
    dt = mybir.dt
    act = mybir.ActivationFunctionType
    X = mybir.AxisListType.X
    alu = mybir.AluOpType

    # enc interleaved: [128, RC, NH]; element (p, r, c) = enc[r, c*128+p]
    enck_d = nc.dram_tensor("enck", [128, RC, NH], dt.bfloat16, kind="ExternalInput")
    # W interleaved: [128, NH, nnz]; element (p, c, n) = W[c*128+p, n], fp8
    wk_d = nc.dram_tensor("wk", [128, NH, nnz], dt.float8e3, kind="ExternalInput")
    # out packed: [128, NB, 96]; element (p, b, m) = out[b*128+p, m]
    out_d = nc.dram_tensor("out", [128, NB, N_PHONEME], dt.float32, kind="ExternalOutput")

    with ExitStack() as ctx:
        tc = ctx.enter_context(tile.TileContext(nc))
        wpool = ctx.enter_context(tc.tile_pool(name="wpool", bufs=1))
        epool = ctx.enter_context(tc.tile_pool(name="epool", bufs=1))
        # 3 double-bank megatile accumulators + 2 single-bank ones for the
        # last megatile (separate tiles per 128-row block there, so its r0
        # postlude reads never alias r1's accumulation in the hazard
        # tracker - a shared tile serializes the PE behind the DVE)
        ppool = ctx.enter_context(tc.tile_pool(name="ppool", bufs=3, space="PSUM"))
        ppool2 = ctx.enter_context(tc.tile_pool(name="ppool2", bufs=2, space="PSUM"))
        spool = ctx.enter_context(tc.tile_pool(name="spool", bufs=3))
        opool = ctx.enter_context(tc.tile_pool(name="opool", bufs=1))

        # Loads, strictly ordered on the Sync queue (the stream is
        # bandwidth-bound, so order = priority): W gates the first matmul,
        # then enc pieces sized to stay ahead of the PE.
        wt = wpool.tile([128, NH, nnz], dt.float8e3)
        nc.sync.dma_start(wt[:], wk_d[:])
        et = epool.tile([128, RC, NH], dt.bfloat16)
        for lo, hi in ((0, 128), (128, 256), (256, 512), (512, RC)):
            nc.sync.dma_start(et[:, lo:hi, :], enck_d[:, lo:hi, :])

        # PE warmup: dummy matmuls on zeroed scratch run while the DMAs
        # land, ramping the tensor engine's p-state.  They write the first
        # megatile's PSUM bank; the real accumulation overwrites it.
        wu = wpool.tile([128, 512], dt.bfloat16)
        nc.vector.memset(wu[:], 0.0)
        ps0 = ppool.tile([128, 2, 512], dt.float32, tag="ps")
        # 6 warmups end right as W + the first enc piece land (~11us):
        # fewer leaves an idle gap that resets the PE's p-state ramp
        # (the next ~14 matmuls then run at ~2x duration), more delays
        # the real stream behind the warmup queue.
        for _ in range(6):
            nc.tensor.matmul(ps0[:, 0, :], wu[:, :128], wu[:], start=True, stop=True)

        obuf = opool.tile([128, NB, N_PHONEME], dt.float32)

        def seg_max(ps, rr, nr, tagn):
            """Segment max of PSUM rows `rr`: one strided DVE reduce per
            segment-length group.  (A pairwise tensor_tensor pre-max would
            halve the reduce work, but the DVE can read only ONE operand
            from PSUM per instruction - walrus NCC_IBVF027.)"""
            pmax = spool.tile([128, nr, N_PHONEME], dt.float32, tag=f"pmax{tagn}")
            for (L, nL, coff, ooff) in groups:
                src = ps[:, rr, coff:coff + nL * L].rearrange(
                    "p r (s l) -> p r s l", l=L
                )
                nc.vector.reduce_max(pmax[:, :, ooff:ooff + nL], src, axis=X)
            return pmax

        def postlude_a(ps, rr, blk, nr):
            """Segment max + exp-sum + Ln for `nr` blocks at `blk`; the
            final subs are emitted LATER (postlude_b) so they queue behind
            the next megatile's reduces: a sub waits on this tile's Ln
            (~1us of ScalarE chain), and the DVE executes in order - subs
            emitted eagerly head-of-line block the next reduces."""
            pmax = seg_max(ps, rr, nr, nr if nr == 2 else blk)
            ex = spool.tile([128, nr, N_PHONEME], dt.float32, tag=f"ex{blk}")
            se = spool.tile([128, nr], dt.float32, tag=f"se{blk}")
            for k in range(nr):
                # exp(scale*x); the row-sum comes free via the activation
                # accumulator (1/sqrt(H) lives here, not in fp8 W)
                nc.scalar.activation(ex[:, k, :], pmax[:, k, :], act.Exp,
                                     scale=SCALE, accum_out=se[:, k:k + 1])
            lse = spool.tile([128, nr], dt.float32, tag=f"lse{blk}")
            nc.scalar.activation(lse[:], se[:], act.Ln)
            return pmax, lse

        def postlude_b(state, blk, nr):
            pmax, lse = state
            for k in range(nr):
                # out = scale*pmax - lse, fused in one DVE op
                nc.vector.tensor_scalar(
                    obuf[:, blk + k, :], pmax[:, k, :],
                    SCALE, lse[:, k:k + 1], op0=alu.mult, op1=alu.subtract,
                )

        def block_matmuls(ps_row, blk):
            row0 = blk * 128
            for c in range(NH):
                nc.tensor.matmul(
                    ps_row[:, :nnz],
                    et[:, row0:row0 + 128, c],
                    wt[:, c, :],
                    start=(c == 0),
                    stop=(c == NH - 1),
                )

        states = []
        for mt in range(NMT - 1):
            ps = ps0 if mt == 0 else ppool.tile([128, 2, 512], dt.float32, tag="ps")
            for r in range(2):
                block_matmuls(ps[:, r, :], mt * 2 + r)
            states.append(postlude_a(ps, slice(0, 2), mt * 2, 2))
            if mt > 0:
                postlude_b(states[mt - 1], (mt - 1) * 2, 2)
        # last megatile: one single-bank tile per block so neither block's
        # postlude aliases the other's accumulation in the hazard tracker
        psr0 = ppool2.tile([128, 1, 512], dt.float32, tag="psr")
        psr1 = ppool2.tile([128, 1, 512], dt.float32, tag="psr")
        block_matmuls(psr0[:, 0, :], NB - 2)
        st6 = postlude_a(psr0, slice(0, 1), NB - 2, 1)
        block_matmuls(psr1[:, 0, :], NB - 1)
        st7 = postlude_a(psr1, slice(0, 1), NB - 1, 1)
        postlude_b(states[NMT - 2], (NMT - 2) * 2, 2)
        # blocks 0-5 stored while the tail drains
        nc.sync.dma_start(out_d[:, :6, :], obuf[:, :6, :])
        postlude_b(st6, NB - 2, 1)
        postlude_b(st7, NB - 1, 1)
        # final piece: blocks 6-7, split by partition halves onto two
        # queues (sync + scalar) so the packets can spread across engines
        nc.sync.dma_start(out_d[:64, 6:, :], obuf[:64, 6:, :])
        nc.scalar.dma_start(out_d[64:, 6:, :], obuf[64:, 6:, :])

    nc.compile()
    return nc


_CACHE: dict = {}


def _get_compiled(mapping: np.ndarray):
    key = mapping.astype(np.float32).tobytes()
    if _CACHE.get("key") != key:
        col_ids, groups, perm = _structure(mapping)
        nc = _build_program(len(col_ids), groups)
        _CACHE.update(key=key, col_ids=col_ids, groups=groups, perm=perm, nc=nc)
    return _CACHE["nc"], _CACHE["col_ids"], _CACHE["perm"]


def _prep_in_maps(enc_output, feature2phone, col_ids):
    wg = feature2phone.astype(np.float32)[:, col_ids].astype(F8)
    # [H, nnz] -> [128, NH, nnz]
    wk = np.ascontiguousarray(wg.reshape(NH, 128, -1).transpose(1, 0, 2))
    # enc [ROWS, H] -> [128, ROWS, NH]
    e3 = enc_output.astype(BF16).reshape(ROWS, NH, 128)
    enck = np.ascontiguousarray(e3.transpose(2, 0, 1))
    in_maps = []
    for c in range(N_CORES):
        in_maps.append({
            "enck": np.ascontiguousarray(enck[:, c * RC:(c + 1) * RC, :]),
            "wk": wk,
        })
    return in_maps


def run_device(enc_output, feature2phone, mapping, trace=False, **kw):
    """Build/compile (cached), run on the 8 cores, return (output, BassKernelResults)."""
    enc_output = np.asarray(enc_output)
    feature2phone = np.asarray(feature2phone)
    mapping = np.asarray(mapping)
    nc, col_ids, perm = _get_compiled(mapping)
    in_maps = _prep_in_maps(enc_output, feature2phone, col_ids)
    res = run_bass_kernel_spmd(
        nc, in_maps, core_ids=list(range(N_CORES)), trace=trace, **kw
    )
    # device out [128, NB, 96] packed -> rows b*128+p
    dev = np.concatenate(
        [res.results[c]["out"].transpose(1, 0, 2).reshape(RC, N_PHONEME)
         for c in range(N_CORES)],
        axis=0,
    )
    out = np.empty_like(dev)
    out[:, perm] = dev
    return out.reshape(B, T, N_PHONEME).astype(np.float32), res


def kernel(enc_output, feature2phone, mapping):
    out, _ = run_device(enc_output, feature2phone, mapping)
    return out


# revision 15
# speedup vs baseline: 1.0727x; 1.0044x over previous
"""Trainium2 Bass kernel for CompositionalPhoneticsModel (segment_reduce).

Computation (reference):
    phone   = einsum('bth,hp->btp', enc_output, feature2phone) / sqrt(H)
    allo    = where(mapping>0, phone[:,:,None,:]*mapping, -inf)   # mapping is 0/1
    phoneme = max(allo, axis=-1)                                  # masked segment max
    out     = log_softmax(phoneme, axis=2)

Device strategy (8 NeuronCores, data-parallel over the B*T=8192 rows):
  * Host gathers feature2phone columns into segment-contiguous order
    (phones in 2 segments get duplicated columns; NNZ ~ 506) and sorts
    segments by length so the per-segment max is a handful of strided DVE
    reduce_max ops.  Host un-permutes the output columns at the end.
  * W ships as fp8 e3m4 (range +-15.5, 4 mantissa bits: ~2x the quantization
    noise of bf16's 8 bits on N(0,1) data but only ~0.8% of the output error
    budget).  The 1/sqrt(H) scale is NOT folded in (it would push W into the
    e3m4 denormal range); it rides the Exp activation's scale input and a
    fused MULT in the final tensor_scalar.  enc stays bf16 - its DMA
    overlaps the matmul stream anyway, so quantizing it buys nothing.
  * DMA on this part is bandwidth-bound at ~287 GB/s once packets are
    >=2KB/partition, with ~2.4us from program start to first byte.  The
    first matmul gates on W + enc block 0, so W (0.32MB in fp8) goes first,
    then enc in pieces sized so the stream stays ahead of the PE: blocks
    [0], [1], [2-3], [4-7].  Output: blocks 0-5 stored during the last
    megatile's compute; the last 2 blocks split by partition halves across
    two queues right after their log-softmax.
  * Postlude per 256-row megatile: batched strided segment-max reduces
    (DVE), per-128-row Exp on ScalarE with the row-sum from the activation
    accumulator, one Ln, and a fused (x*scale - lse) DVE tensor_scalar.
    The last megatile runs everything per 128-row block instead, shortening
    the serial tail after the final matmul.
"""

from contextlib import ExitStack

import numpy as np
import ml_dtypes

import concourse.bass as bass
import concourse.bacc as bacc
import concourse.tile as tile
from concourse import mybir
from concourse.bass_utils import run_bass_kernel_spmd

B, T, H = 8, 1024, 640
N_PHONEME, N_PHONE = 96, 230
N_CORES = 8
ROWS = B * T
RC = ROWS // N_CORES          # rows per core
NH = H // 128                 # contraction chunks
NB = RC // 128                # 128-row blocks per core
NMT = NB // 2                 # megatiles (2 blocks each)
BF16 = ml_dtypes.bfloat16
F8 = ml_dtypes.float8_e3m4
SCALE = float(1.0 / np.sqrt(np.float32(H)))


def _structure(mapping: np.ndarray):
    """Segment-contiguous gather order, grouped by segment length (desc).

    Returns (col_ids, groups, perm):
      col_ids: phone index feeding each device matmul column (len NNZ)
      groups:  list of (L, nL, col_off, out_off) — nL segments of length L
               occupy matmul cols [col_off, col_off+nL*L) and device output
               cols [out_off, out_off+nL)
      perm:    perm[j] = original phoneme id of device output column j
    """
    segs = [np.nonzero(mapping[m] > 0)[0] for m in range(N_PHONEME)]
    assert min(len(s) for s in segs) >= 1
    # pad segment lengths up to even targets (repeating a member doesn't
    # change the max): fewer distinct lengths -> fewer DVE reduce ops.
    # Only worthwhile while the matmul width stays within one PSUM bank.
    padded = []
    for s in segs:
        t = ((len(s) + 1) // 2) * 2
        padded.append(np.concatenate([s, np.full(t - len(s), s[0], s.dtype)]))
    if sum(len(s) for s in padded) <= 512:
        segs = padded
    lengths = np.array([len(s) for s in segs])
    order = np.argsort(-lengths, kind="stable")
    col_ids, groups, perm = [], [], []
    i = 0
    while i < N_PHONEME:
        L = int(lengths[order[i]])
        j = i
        while j < N_PHONEME and lengths[order[j]] == L:
            j += 1
        groups.append((L, j - i, len(col_ids), i))
        for k in range(i, j):
            m = int(order[k])
            col_ids.extend(segs[m].tolist())
            perm.append(m)
        i = j
    return np.array(col_ids, dtype=np.int64), groups, np.array(perm, dtype=np.int64)


def _patch_act_tables():
    """Make Exp and Ln resolve to the same activation-table set.

    bacc's insert_act_table_loads models a single table slot, so a kernel
    alternating Exp/Ln reloads a 1.3us table on every transition.  act_info
    has a joint set ('natural_log_exp_and_others') containing both; keep the
    set list's order/indices intact but strip Exp/Ln from the other sets so
    the pass picks the joint set for both and emits a single load.
    """
    if getattr(bacc, "_act_tables_patched", False):
        return
    from concourse import hw_specs
    orig = hw_specs.get_activation_tables
    act = mybir.ActivationFunctionType

    def patched(module_arch):
        tabs = orig(module_arch)
        joint = [k for k, v in tabs.items() if act.Exp in v and act.Ln in v]
        if not joint:
            return tabs
        j = joint[0]
        return {
            k: (v if k == j else (v - {act.Exp, act.Ln}))
            for k, v in tabs.items()
        }

    bacc.get_activation_tables = patched
    bacc._act_tables_patched = True


def _build_program(nnz: int, groups):
    """Build + compile the per-core Bass program. Returns the Bacc object."""
    _patch_act_tables()
    nc = bacc.Bacc("TRN2", target_bir_lowering=False, debug=False)
    dt = mybir.dt
    act = mybir.ActivationFunctionType
    X = mybir.AxisListType.X
    alu = mybir.AluOpType

    # enc interleaved: [128, RC, NH]; element (p, r, c) = enc[r, c*128+p]
    enck_d = nc.dram_tensor("enck", [128, RC, NH], dt.float8e3, kind="ExternalInput")
    # W interleaved: [128, NH, nnz]; element (p, c, n) = W[c*128+p, n], fp8
    wk_d = nc.dram_tensor("wk", [128, NH, nnz], dt.float8e3, kind="ExternalInput")
    # out packed: [128, NB, 96]; element (p, b, m) = out[b*128+p, m]
    out_d = nc.dram_tensor("out", [128, NB, N_PHONEME], dt.float32, kind="ExternalOutput")

    with ExitStack() as ctx:
        tc = ctx.enter_context(tile.TileContext(nc))
        wpool = ctx.enter_context(tc.tile_pool(name="wpool", bufs=1))
        epool = ctx.enter_context(tc.tile_pool(name="epool", bufs=1))
        # 3 double-bank megatile accumulators + 2 single-bank ones for the
        # last megatile (separate tiles per 128-row block there, so its r0
        # postlude reads never alias r1's accumulation in the hazard
        # tracker - a shared tile serializes the PE behind the DVE)
        ppool = ctx.enter_context(tc.tile_pool(name="ppool", bufs=3, space="PSUM"))
        ppool2 = ctx.enter_context(tc.tile_pool(name="ppool2", bufs=2, space="PSUM"))
        spool = ctx.enter_context(tc.tile_pool(name="spool", bufs=3))
        opool = ctx.enter_context(tc.tile_pool(name="opool", bufs=1))

        # Loads, strictly ordered on the Sync queue (the stream is
        # bandwidth-bound, so order = priority): W gates the first matmul,
        # then enc pieces sized to stay ahead of the PE.
        wt = wpool.tile([128, NH, nnz], dt.float8e3)
        nc.sync.dma_start(wt[:], wk_d[:])
        et = epool.tile([128, RC, NH], dt.float8e3)
        for lo, hi in ((0, 128), (128, 256), (256, 512), (512, RC)):
            nc.sync.dma_start(et[:, lo:hi, :], enck_d[:, lo:hi, :])

        # PE warmup: dummy matmuls on zeroed scratch run while the DMAs
        # land, ramping the tensor engine's p-state.  They write the first
        # megatile's PSUM bank; the real accumulation overwrites it.
        wu = wpool.tile([128, 512], dt.bfloat16)
        nc.vector.memset(wu[:], 0.0)
        ps0 = ppool.tile([128, 2, 512], dt.float32, tag="ps")
        # 6 warmups end right as W + the first enc piece land (~11us):
        # fewer leaves an idle gap that resets the PE's p-state ramp
        # (the next ~14 matmuls then run at ~2x duration), more delays
        # the real stream behind the warmup queue.
        for _ in range(6):
            nc.tensor.matmul(ps0[:, 0, :], wu[:, :128], wu[:], start=True, stop=True)

        obuf = opool.tile([128, NB, N_PHONEME], dt.float32)

        def seg_max(ps, rr, nr, tagn):
            """Segment max of PSUM rows `rr`: one strided DVE reduce per
            segment-length group.  (A pairwise tensor_tensor pre-max would
            halve the reduce work, but the DVE can read only ONE operand
            from PSUM per instruction - walrus NCC_IBVF027.)"""
            pmax = spool.tile([128, nr, N_PHONEME], dt.float32, tag=f"pmax{tagn}")
            for (L, nL, coff, ooff) in groups:
                src = ps[:, rr, coff:coff + nL * L].rearrange(
                    "p r (s l) -> p r s l", l=L
                )
                nc.vector.reduce_max(pmax[:, :, ooff:ooff + nL], src, axis=X)
            return pmax

        def postlude_a(ps, rr, blk, nr):
            """Segment max + exp-sum + Ln for `nr` blocks at `blk`; the
            final subs are emitted LATER (postlude_b) so they queue behind
            the next megatile's reduces: a sub waits on this tile's Ln
            (~1us of ScalarE chain), and the DVE executes in order - subs
            emitted eagerly head-of-line block the next reduces."""
            pmax = seg_max(ps, rr, nr, nr if nr == 2 else blk)
            ex = spool.tile([128, nr, N_PHONEME], dt.float32, tag=f"ex{blk}")
            se = spool.tile([128, nr], dt.float32, tag=f"se{blk}")
            for k in range(nr):
                # exp(scale*x); the row-sum comes free via the activation
                # accumulator (1/sqrt(H) lives here, not in fp8 W)
                nc.scalar.activation(ex[:, k, :], pmax[:, k, :], act.Exp,
                                     scale=SCALE, accum_out=se[:, k:k + 1])
            lse = spool.tile([128, nr], dt.float32, tag=f"lse{blk}")
            nc.scalar.activation(lse[:], se[:], act.Ln)
            return pmax, lse

        def postlude_b(state, blk, nr):
            pmax, lse = state
            for k in range(nr):
                # out = scale*pmax - lse, fused in one DVE op
                nc.vector.tensor_scalar(
                    obuf[:, blk + k, :], pmax[:, k, :],
                    SCALE, lse[:, k:k + 1], op0=alu.mult, op1=alu.subtract,
                )

        def block_matmuls(ps_row, blk):
            row0 = blk * 128
            for c in range(NH):
                nc.tensor.matmul(
                    ps_row[:, :nnz],
                    et[:, row0:row0 + 128, c],
                    wt[:, c, :],
                    start=(c == 0),
                    stop=(c == NH - 1),
                )

        states = []
        for mt in range(NMT - 1):
            ps = ps0 if mt == 0 else ppool.tile([128, 2, 512], dt.float32, tag="ps")
            for r in range(2):
                block_matmuls(ps[:, r, :], mt * 2 + r)
            states.append(postlude_a(ps, slice(0, 2), mt * 2, 2))
            if mt > 0:
                postlude_b(states[mt - 1], (mt - 1) * 2, 2)
        # last megatile: one single-bank tile per block so neither block's
        # postlude aliases the other's accumulation in the hazard tracker
        psr0 = ppool2.tile([128, 1, 512], dt.float32, tag="psr")
        psr1 = ppool2.tile([128, 1, 512], dt.float32, tag="psr")
        block_matmuls(psr0[:, 0, :], NB - 2)
        st6 = postlude_a(psr0, slice(0, 1), NB - 2, 1)
        block_matmuls(psr1[:, 0, :], NB - 1)
        st7 = postlude_a(psr1, slice(0, 1), NB - 1, 1)
        postlude_b(states[NMT - 2], (NMT - 2) * 2, 2)
        postlude_b(st6, NB - 2, 1)
        # blocks 0-6 stored while block 7's postlude drains
        nc.sync.dma_start(out_d[:, :7, :], obuf[:, :7, :])
        postlude_b(st7, NB - 1, 1)
        # final piece: block 7 only, split by partition halves onto two
        # queues (sync + scalar) so the packets can spread across engines
        nc.sync.dma_start(out_d[:64, 7:, :], obuf[:64, 7:, :])
        nc.scalar.dma_start(out_d[64:, 7:, :], obuf[64:, 7:, :])

    nc.compile()
    return nc


_CACHE: dict = {}


def _get_compiled(mapping: np.ndarray):
    key = mapping.astype(np.float32).tobytes()
    if _CACHE.get("key") != key:
        col_ids, groups, perm = _structure(mapping)
        nc = _build_program(len(col_ids), groups)
        _CACHE.update(key=key, col_ids=col_ids, groups=groups, perm=perm, nc=nc)
    return _CACHE["nc"], _CACHE["col_ids"], _CACHE["perm"]


def _prep_in_maps(enc_output, feature2phone, col_ids):
    wg = feature2phone.astype(np.float32)[:, col_ids].astype(F8)
    # [H, nnz] -> [128, NH, nnz]
    wk = np.ascontiguousarray(wg.reshape(NH, 128, -1).transpose(1, 0, 2))
    # enc [ROWS, H] -> [128, ROWS, NH]
    e3 = enc_output.astype(F8).reshape(ROWS, NH, 128)
    enck = np.ascontiguousarray(e3.transpose(2, 0, 1))
    in_maps = []
    for c in range(N_CORES):
        in_maps.append({
            "enck": np.ascontiguousarray(enck[:, c * RC:(c + 1) * RC, :]),
            "wk": wk,
        })
    return in_maps


def run_device(enc_output, feature2phone, mapping, trace=False, **kw):
    """Build/compile (cached), run on the 8 cores, return (output, BassKernelResults)."""
    enc_output = np.asarray(enc_output)
    feature2phone = np.asarray(feature2phone)
    mapping = np.asarray(mapping)
    nc, col_ids, perm = _get_compiled(mapping)
    in_maps = _prep_in_maps(enc_output, feature2phone, col_ids)
    res = run_bass_kernel_spmd(
        nc, in_maps, core_ids=list(range(N_CORES)), trace=trace, **kw
    )
    # device out [128, NB, 96] packed -> rows b*128+p
    dev = np.concatenate(
        [res.results[c]["out"].transpose(1, 0, 2).reshape(RC, N_PHONEME)
         for c in range(N_CORES)],
        axis=0,
    )
    out = np.empty_like(dev)
    out[:, perm] = dev
    return out.reshape(B, T, N_PHONEME).astype(np.float32), res


def kernel(enc_output, feature2phone, mapping):
    out, _ = run_device(enc_output, feature2phone, mapping)
    return out


# revision 16
# speedup vs baseline: 1.0970x; 1.0227x over previous
"""Trainium2 Bass kernel for CompositionalPhoneticsModel (segment_reduce).

Computation (reference):
    phone   = einsum('bth,hp->btp', enc_output, feature2phone) / sqrt(H)
    allo    = where(mapping>0, phone[:,:,None,:]*mapping, -inf)   # mapping is 0/1
    phoneme = max(allo, axis=-1)                                  # masked segment max
    out     = log_softmax(phoneme, axis=2)

Device strategy (8 NeuronCores, data-parallel over the B*T=8192 rows):
  * Host gathers feature2phone columns into segment-contiguous order
    (phones in 2 segments get duplicated columns; NNZ ~ 506) and sorts
    segments by length so the per-segment max is a handful of strided DVE
    reduce_max ops.  Host un-permutes the output columns at the end.
  * W ships as fp8 e3m4 (range +-15.5, 4 mantissa bits: ~2x the quantization
    noise of bf16's 8 bits on N(0,1) data but only ~0.8% of the output error
    budget).  The 1/sqrt(H) scale is NOT folded in (it would push W into the
    e3m4 denormal range); it rides the Exp activation's scale input and a
    fused MULT in the final tensor_scalar.  enc stays bf16 - its DMA
    overlaps the matmul stream anyway, so quantizing it buys nothing.
  * DMA on this part is bandwidth-bound at ~287 GB/s once packets are
    >=2KB/partition, with ~2.4us from program start to first byte.  The
    first matmul gates on W + enc block 0, so W (0.32MB in fp8) goes first,
    then enc in pieces sized so the stream stays ahead of the PE: blocks
    [0], [1], [2-3], [4-7].  Output: blocks 0-5 stored during the last
    megatile's compute; the last 2 blocks split by partition halves across
    two queues right after their log-softmax.
  * Postlude per 256-row megatile: batched strided segment-max reduces
    (DVE), per-128-row Exp on ScalarE with the row-sum from the activation
    accumulator, one Ln, and a fused (x*scale - lse) DVE tensor_scalar.
    The last megatile runs everything per 128-row block instead, shortening
    the serial tail after the final matmul.
"""

from contextlib import ExitStack

import numpy as np
import ml_dtypes

import concourse.bass as bass
import concourse.bacc as bacc
import concourse.tile as tile
from concourse import mybir
from concourse.bass_utils import run_bass_kernel_spmd

B, T, H = 8, 1024, 640
N_PHONEME, N_PHONE = 96, 230
N_CORES = 8
ROWS = B * T
RC = ROWS // N_CORES          # rows per core
NH = H // 128                 # contraction chunks
NB = RC // 128                # 128-row blocks per core
NMT = NB // 2                 # megatiles (2 blocks each)
BF16 = ml_dtypes.bfloat16
F8 = ml_dtypes.float8_e3m4
SCALE = float(1.0 / np.sqrt(np.float32(H)))


def _structure(mapping: np.ndarray):
    """Segment-contiguous gather order, grouped by segment length (desc).

    Returns (col_ids, groups, perm):
      col_ids: phone index feeding each device matmul column (len NNZ)
      groups:  list of (L, nL, col_off, out_off) — nL segments of length L
               occupy matmul cols [col_off, col_off+nL*L) and device output
               cols [out_off, out_off+nL)
      perm:    perm[j] = original phoneme id of device output column j
    """
    segs = [np.nonzero(mapping[m] > 0)[0] for m in range(N_PHONEME)]
    assert min(len(s) for s in segs) >= 1
    # pad segment lengths up to even targets (repeating a member doesn't
    # change the max): fewer distinct lengths -> fewer DVE reduce ops.
    # Only worthwhile while the matmul width stays within one PSUM bank.
    padded = []
    for s in segs:
        t = ((len(s) + 1) // 2) * 2
        padded.append(np.concatenate([s, np.full(t - len(s), s[0], s.dtype)]))
    if sum(len(s) for s in padded) <= 512:
        segs = padded
    lengths = np.array([len(s) for s in segs])
    order = np.argsort(-lengths, kind="stable")
    col_ids, groups, perm = [], [], []
    i = 0
    while i < N_PHONEME:
        L = int(lengths[order[i]])
        j = i
        while j < N_PHONEME and lengths[order[j]] == L:
            j += 1
        groups.append((L, j - i, len(col_ids), i))
        for k in range(i, j):
            m = int(order[k])
            col_ids.extend(segs[m].tolist())
            perm.append(m)
        i = j
    return np.array(col_ids, dtype=np.int64), groups, np.array(perm, dtype=np.int64)


def _patch_act_tables():
    """Make Exp and Ln resolve to the same activation-table set.

    bacc's insert_act_table_loads models a single table slot, so a kernel
    alternating Exp/Ln reloads a 1.3us table on every transition.  act_info
    has a joint set ('natural_log_exp_and_others') containing both; keep the
    set list's order/indices intact but strip Exp/Ln from the other sets so
    the pass picks the joint set for both and emits a single load.
    """
    if getattr(bacc, "_act_tables_patched", False):
        return
    from concourse import hw_specs
    orig = hw_specs.get_activation_tables
    act = mybir.ActivationFunctionType

    def patched(module_arch):
        tabs = orig(module_arch)
        joint = [k for k, v in tabs.items() if act.Exp in v and act.Ln in v]
        if not joint:
            return tabs
        j = joint[0]
        return {
            k: (v if k == j else (v - {act.Exp, act.Ln}))
            for k, v in tabs.items()
        }

    bacc.get_activation_tables = patched
    bacc._act_tables_patched = True


def _build_program(nnz: int, groups):
    """Build + compile the per-core Bass program. Returns the Bacc object."""
    _patch_act_tables()
    nc = bacc.Bacc("TRN2", target_bir_lowering=False, debug=False)
    dt = mybir.dt
    act = mybir.ActivationFunctionType
    X = mybir.AxisListType.X
    alu = mybir.AluOpType

    # enc interleaved: [128, RC, NH]; element (p, r, c) = enc[r, c*128+p]
    enck_d = nc.dram_tensor("enck", [128, RC, NH], dt.float8e3, kind="ExternalInput")
    # W interleaved: [128, NH, nnz]; element (p, c, n) = W[c*128+p, n], fp8
    wk_d = nc.dram_tensor("wk", [128, NH, nnz], dt.float8e3, kind="ExternalInput")
    # out packed: [128, NB, 96]; element (p, b, m) = out[b*128+p, m]
    out_d = nc.dram_tensor("out", [128, NB, N_PHONEME], dt.float32, kind="ExternalOutput")

    with ExitStack() as ctx:
        tc = ctx.enter_context(tile.TileContext(nc))
        wpool = ctx.enter_context(tc.tile_pool(name="wpool", bufs=1))
        epool = ctx.enter_context(tc.tile_pool(name="epool", bufs=1))
        # 3 double-bank megatile accumulators + 2 single-bank ones for the
        # last megatile (separate tiles per 128-row block there, so its r0
        # postlude reads never alias r1's accumulation in the hazard
        # tracker - a shared tile serializes the PE behind the DVE)
        ppool = ctx.enter_context(tc.tile_pool(name="ppool", bufs=3, space="PSUM"))
        ppool2 = ctx.enter_context(tc.tile_pool(name="ppool2", bufs=2, space="PSUM"))
        spool = ctx.enter_context(tc.tile_pool(name="spool", bufs=3))
        opool = ctx.enter_context(tc.tile_pool(name="opool", bufs=1))

        # Loads, strictly ordered on the Sync queue (the stream is
        # bandwidth-bound, so order = priority): W gates the first matmul,
        # then enc pieces sized to stay ahead of the PE.
        wt = wpool.tile([128, NH, nnz], dt.float8e3)
        nc.sync.dma_start(wt[:], wk_d[:])
        et = epool.tile([128, RC, NH], dt.float8e3)
        for lo, hi in ((0, 128), (128, 256), (256, 512), (512, RC)):
            nc.sync.dma_start(et[:, lo:hi, :], enck_d[:, lo:hi, :])

        # PE warmup: dummy matmuls on zeroed scratch run while the DMAs
        # land, ramping the tensor engine's p-state.  They write the first
        # megatile's PSUM bank; the real accumulation overwrites it.
        wu = wpool.tile([128, 512], dt.bfloat16)
        nc.vector.memset(wu[:], 0.0)
        ps0 = ppool.tile([128, 2, 512], dt.float32, tag="ps")
        # 6 warmups end right as W + the first enc piece land (~11us):
        # fewer leaves an idle gap that resets the PE's p-state ramp
        # (the next ~14 matmuls then run at ~2x duration), more delays
        # the real stream behind the warmup queue.
        for _ in range(6):
            nc.tensor.matmul(ps0[:, 0, :], wu[:, :128], wu[:], start=True, stop=True)

        obuf = opool.tile([128, NB, N_PHONEME], dt.float32)

        def seg_max(ps, rr, nr, tagn):
            """Segment max of PSUM rows `rr`: one strided DVE reduce per
            segment-length group.  (A pairwise tensor_tensor pre-max would
            halve the reduce work, but the DVE can read only ONE operand
            from PSUM per instruction - walrus NCC_IBVF027.)"""
            pmax = spool.tile([128, nr, N_PHONEME], dt.float32, tag=f"pmax{tagn}")
            for (L, nL, coff, ooff) in groups:
                src = ps[:, rr, coff:coff + nL * L].rearrange(
                    "p r (s l) -> p r s l", l=L
                )
                nc.vector.reduce_max(pmax[:, :, ooff:ooff + nL], src, axis=X)
            return pmax

        def postlude_a(ps, rr, blk, nr):
            """Segment max + exp-sum + Ln for `nr` blocks at `blk`; the
            final subs are emitted LATER (postlude_b) so they queue behind
            the next megatile's reduces: a sub waits on this tile's Ln
            (~1us of ScalarE chain), and the DVE executes in order - subs
            emitted eagerly head-of-line block the next reduces."""
            pmax = seg_max(ps, rr, nr, nr if nr == 2 else blk)
            ex = spool.tile([128, nr, N_PHONEME], dt.float32, tag=f"ex{blk}")
            se = spool.tile([128, nr], dt.float32, tag=f"se{blk}")
            for k in range(nr):
                # exp(scale*x); the row-sum comes free via the activation
                # accumulator (1/sqrt(H) lives here, not in fp8 W)
                nc.scalar.activation(ex[:, k, :], pmax[:, k, :], act.Exp,
                                     scale=SCALE, accum_out=se[:, k:k + 1])
            lse = spool.tile([128, nr], dt.float32, tag=f"lse{blk}")
            nc.scalar.activation(lse[:], se[:], act.Ln)
            return pmax, lse

        def postlude_b(state, blk, nr):
            pmax, lse = state
            for k in range(nr):
                # out = scale*pmax - lse, fused in one DVE op
                nc.vector.tensor_scalar(
                    obuf[:, blk + k, :], pmax[:, k, :],
                    SCALE, lse[:, k:k + 1], op0=alu.mult, op1=alu.subtract,
                )

        def block_matmuls(ps_row, blk):
            row0 = blk * 128
            for c in range(NH):
                nc.tensor.matmul(
                    ps_row[:, :nnz],
                    et[:, row0:row0 + 128, c],
                    wt[:, c, :],
                    start=(c == 0),
                    stop=(c == NH - 1),
                )

        states = []
        for mt in range(NMT - 1):
            ps = ps0 if mt == 0 else ppool.tile([128, 2, 512], dt.float32, tag="ps")
            for r in range(2):
                block_matmuls(ps[:, r, :], mt * 2 + r)
            states.append(postlude_a(ps, slice(0, 2), mt * 2, 2))
            if mt > 0:
                postlude_b(states[mt - 1], (mt - 1) * 2, 2)
        # last megatile: one single-bank tile per block so neither block's
        # postlude aliases the other's accumulation in the hazard tracker
        psr0 = ppool2.tile([128, 1, 512], dt.float32, tag="psr")
        psr1 = ppool2.tile([128, 1, 512], dt.float32, tag="psr")
        block_matmuls(psr0[:, 0, :], NB - 2)
        st6 = postlude_a(psr0, slice(0, 1), NB - 2, 1)
        block_matmuls(psr1[:, 0, :], NB - 1)
        st7 = postlude_a(psr1, slice(0, 1), NB - 1, 1)
        postlude_b(states[NMT - 2], (NMT - 2) * 2, 2)
        # blocks 0-5 stored while the tail drains
        nc.sync.dma_start(out_d[:, :6, :], obuf[:, :6, :])
        postlude_b(st6, NB - 2, 1)
        nc.sync.dma_start(out_d[:, 6:7, :], obuf[:, 6:7, :])
        postlude_b(st7, NB - 1, 1)
        # final piece: block 7 only, split by partition halves onto two
        # queues (sync + scalar) so the packets can spread across engines
        nc.sync.dma_start(out_d[:64, 7:, :], obuf[:64, 7:, :])
        nc.scalar.dma_start(out_d[64:, 7:, :], obuf[64:, 7:, :])

    nc.compile()
    return nc


_CACHE: dict = {}


def _get_compiled(mapping: np.ndarray):
    key = mapping.astype(np.float32).tobytes()
    if _CACHE.get("key") != key:
        col_ids, groups, perm = _structure(mapping)
        nc = _build_program(len(col_ids), groups)
        _CACHE.update(key=key, col_ids=col_ids, groups=groups, perm=perm, nc=nc)
    return _CACHE["nc"], _CACHE["col_ids"], _CACHE["perm"]


def _prep_in_maps(enc_output, feature2phone, col_ids):
    wg = feature2phone.astype(np.float32)[:, col_ids].astype(F8)
    # [H, nnz] -> [128, NH, nnz]
    wk = np.ascontiguousarray(wg.reshape(NH, 128, -1).transpose(1, 0, 2))
    # enc [ROWS, H] -> [128, ROWS, NH]
    e3 = enc_output.astype(F8).reshape(ROWS, NH, 128)
    enck = np.ascontiguousarray(e3.transpose(2, 0, 1))
    in_maps = []
    for c in range(N_CORES):
        in_maps.append({
            "enck": np.ascontiguousarray(enck[:, c * RC:(c + 1) * RC, :]),
            "wk": wk,
        })
    return in_maps


def run_device(enc_output, feature2phone, mapping, trace=False, **kw):
    """Build/compile (cached), run on the 8 cores, return (output, BassKernelResults)."""
    enc_output = np.asarray(enc_output)
    feature2phone = np.asarray(feature2phone)
    mapping = np.asarray(mapping)
    nc, col_ids, perm = _get_compiled(mapping)
    in_maps = _prep_in_maps(enc_output, feature2phone, col_ids)
    res = run_bass_kernel_spmd(
        nc, in_maps, core_ids=list(range(N_CORES)), trace=trace, **kw
    )
    # device out [128, NB, 96] packed -> rows b*128+p
    dev = np.concatenate(
        [res.results[c]["out"].transpose(1, 0, 2).reshape(RC, N_PHONEME)
         for c in range(N_CORES)],
        axis=0,
    )
    out = np.empty_like(dev)
    out[:, perm] = dev
    return out.reshape(B, T, N_PHONEME).astype(np.float32), res


def kernel(enc_output, feature2phone, mapping):
    out, _ = run_device(enc_output, feature2phone, mapping)
    return out
